# revision 12
# baseline (speedup 1.0000x reference)
"""Blended-MoE 3-layer MLP (nn_Expert) on 8 Trainium2 NeuronCores.

Math: per layer, y[b,o] = act( sum_e blend[b,e] * (W[e] @ x[b] + B[e])[o] ).
Rewritten as a dense matmul with a per-expert prescale of the activations:
  y[o,b] = act( sum_e sum_k Wf[e*I+k, o] * (blend[e,b] * hT[k,b]) + bias )
where Wf[(e,i), o] = W[e,o,i] and everything is kept transposed on-chip
([feature, batch] layout) so each layer's output feeds the next directly.

Sharding: data-parallel over the batch — 4096 tokens -> 512 per core; the
per-expert weight stacks are replicated. Matmuls run in bf16 (issue interval
215.9ns vs 226.7ns for float32r — the fp32r path pays a fixed ~13ns/MM
instruction tax) with fp32 PSUM accumulation; rel err ~5e-3. PSUM accumulates
over experts and the contraction. Activations are stored 4 k-blocks wide
([128, 2048] bf16) so one DVE prescale op (with a free-dim-broadcast blend
operand, all-bf16 for the 2x 16-bit DVE rate) feeds 4 k-tiles of matmuls.
Weights are pre-split by o-half on the host so every tile DMA is one
contiguous 256KB block.

DMA issue is split across the two hardware-DGE queues: the Sync queue
carries only the weight stream (whose pool anti-dependency waits pace the
prefetch), while the Activation queue carries x/blend/bias loads and the
output stores, so none of those ever sit behind a stalled weight DMA
(head-of-line blocking on the in-order queue was worth ~20us at boot).
"""

import os

import numpy as np
import ml_dtypes

import concourse.bass as bass
import concourse.tile as tile
import concourse.mybir as mybir
from concourse import bacc
from concourse.bass_utils import run_bass_kernel_spmd
from contextlib import ExitStack

dt = mybir.dt
ALU = mybir.AluOpType
ACTF = mybir.ActivationFunctionType

N_CORES = 8
B_FULL = 4096
BC = B_FULL // N_CORES  # 512 tokens per core
E = 8
DIMS = [1024, 2048, 2048, 512]
LAYERS = [  # (I, O, has_elu)
    (1024, 2048, True),
    (2048, 2048, True),
    (2048, 512, False),
]
OH = 1024   # o-columns per half-pass (8 psum banks)
GW = 4      # k-blocks packed per wide activation tile
WIDE = GW * BC

USE_BF16 = os.environ.get("MOE_MM_DT", "bf16") == "bf16"
MM_DT = dt.bfloat16 if USE_BF16 else dt.float32r
MM_NP = ml_dtypes.bfloat16 if USE_BF16 else np.float32

_cache = {}


def _build(with_bias=True):
    nc = bacc.Bacc("TRN2", target_bir_lowering=False, debug=False,
                   num_devices=N_CORES)
    xTw = nc.declare_dram_parameter("xTw", [DIMS[0] // (128 * GW), 128, WIDE],
                                    MM_DT, isOutput=False)
    blT = nc.declare_dram_parameter("blT", [E, BC], MM_DT, isOutput=False)
    brep = nc.declare_dram_parameter("brep", [E, 128, BC], MM_DT, isOutput=False)
    # weights pre-split by o-half on the host so every [128, width] tile DMA
    # reads one fully contiguous block
    wf = [nc.declare_dram_parameter(f"w{l}f",
                                    [max(O // OH, 1), E * I, min(OH, O)],
                                    MM_DT, isOutput=False)
          for l, (I, O, _) in enumerate(LAYERS)]
    bf = [nc.declare_dram_parameter(f"b{l}f", [E, O], MM_DT, isOutput=False)
          for l, (I, O, _) in enumerate(LAYERS)]
    yT = nc.declare_dram_parameter("yT", [DIMS[3], BC], dt.float32, isOutput=True)

    tc = tile.TileContext(nc)
    with tc:
        with ExitStack() as ctx:
            const = ctx.enter_context(tc.tile_pool(name="const", bufs=1))
            act = ctx.enter_context(tc.tile_pool(name="act", bufs=1))
            xpool = ctx.enter_context(tc.tile_pool(name="xpool", bufs=4))
            wpool = ctx.enter_context(tc.tile_pool(name="wpool", bufs=24))
            tpool = ctx.enter_context(tc.tile_pool(name="tpool", bufs=2))
            ypool = ctx.enter_context(tc.tile_pool(name="ypool", bufs=4))
            pp = ctx.enter_context(tc.tile_pool(name="pp", bufs=8, space="PSUM"))

            if with_bias:
                blT_sb = const.tile([E, BC], MM_DT, tag="blT")
                nc.scalar.dma_start(blT_sb[:], blT[:])

            # warm the PE clock gate (HAM) with throwaway matmuls on a
            # memset constant (no DMA dependency) while the first input and
            # weight DMAs are in flight, so the real stream starts at
            # 2.4 GHz; the psum tile recycles into the first half's bank set
            if USE_BF16:
                wsrc = const.tile([E, BC], MM_DT, tag="wsrc")
                nc.vector.memset(wsrc[:], 1.0)
            else:
                wsrc_f = const.tile([E, BC], dt.float32, tag="wsrc_f")
                nc.vector.memset(wsrc_f[:], 1.0)
                wsrc = const.tile([E, BC], MM_DT, tag="wsrc")
                nc.vector.tensor_copy(wsrc[:], wsrc_f[:])
            warm = pp.tile([128, BC], dt.float32, tag="ps")
            for _ in range(10):
                nc.tensor.matmul(warm[:], wsrc[:, :128], wsrc[:],
                                 start=True, stop=True)

            # blend broadcast tiles: expert 0 first on the weight queue (it
            # gates the very first prescale), the rest trickled on the ACT
            # queue so they never queue behind weight DMAs
            brep_sb = [None] * E
            brep_sb[0] = const.tile([128, BC], MM_DT, name="brep0", tag="brep0")
            nc.sync.dma_start(brep_sb[0][:], brep[0, :, :])

            def load_brep_rest():
                for e in range(1, E):
                    t = const.tile([128, BC], MM_DT, name=f"brep{e}",
                                   tag=f"brep{e}")
                    nc.scalar.dma_start(t[:], brep[e, :, :])
                    brep_sb[e] = t

            bf_sb = [None, None, None]

            # layer 0 input, packed 4 k-blocks wide; tile g loaded lazily at
            # first use so startup only waits for tile 0 (which is itself
            # split into chunks so the first prescale can begin early)
            hT = [None] * (DIMS[0] // (128 * GW))

            def load_x(g):
                t = act.tile([128, WIDE], MM_DT, name=f"h0_{g}", tag=f"h0_{g}")
                # 2 chunks spread across DMA engines for latency
                nc.scalar.dma_start(t[:, :WIDE // 2], xTw[g, :, :WIDE // 2])
                nc.scalar.dma_start(t[:, WIDE // 2:], xTw[g, :, WIDE // 2:])
                hT[g] = t

            pending_drain = []  # deferred drain emission from the previous half

            for l, (I, O, has_elu) in enumerate(LAYERS):
                NG = I // (128 * GW)  # wide groups per layer input
                if with_bias:
                    t = const.tile([E, O], MM_DT, tag=f"bf{l}")
                    nc.scalar.dma_start(t[:], bf[l][:])
                    bf_sb[l] = t
                h_next = []
                if has_elu:
                    for g in range(O * BC // (128 * WIDE)):
                        h_next.append(act.tile([128, WIDE], MM_DT,
                                               name=f"h{l + 1}_{g}",
                                               tag=f"h{l + 1}_{g}"))
                for half_start in range(0, O, OH):
                    width = min(OH, O - half_start)
                    n_ot = width // 128
                    # open accumulation groups with the bias matmul (K=8);
                    # without bias the first weight matmul opens the group
                    ps = []
                    for j in range(n_ot):
                        p = pp.tile([128, BC], dt.float32, tag="ps")
                        if with_bias:
                            nc.tensor.matmul(
                                p[:],
                                bf_sb[l][:, half_start + j * 128:
                                         half_start + (j + 1) * 128],
                                blT_sb[:],
                                start=True, stop=False)
                        ps.append(p)
                    # stream wide k-groups x experts: g-outer so the second
                    # x group / later brep tiles aren't needed until hundreds
                    # of matmuls in (the e-outer order starved the boot)
                    for g in range(NG):
                        for e in range(E):
                            if l == 0 and half_start == 0 and e == 0 and g == 0:
                                # boot: cold DMA completion latency is ~3.5us,
                                # so issue every boot-critical transfer up
                                # front, ordered by when the pipeline needs
                                # it (x_c0, first weights, then alternating),
                                # and bridge the wait with the warmup matmuls
                                t0 = act.tile([128, WIDE], MM_DT,
                                              name="h0_0", tag="h0_0")
                                hT[0] = t0
                                xp = xpool.tile([128, WIDE], MM_DT, tag="xp")
                                wts0 = [wpool.tile([128, width], MM_DT,
                                                   name=f"wtb{c}", tag="wt")
                                        for c in range(GW)]
                                nc.sync.dma_start(t0[:, :BC], xTw[0, :, :BC])
                                hw = width // 2
                                nc.sync.dma_start(
                                    wts0[0][:, :hw], wf[0][0, :128, :hw])
                                nc.sync.dma_start(
                                    wts0[0][:, hw:], wf[0][0, :128, hw:])
                                nc.sync.dma_start(t0[:, BC:2 * BC],
                                                  xTw[0, :, BC:2 * BC])
                                nc.sync.dma_start(
                                    wts0[1][:], wf[0][0, 128:256, :])
                                nc.sync.dma_start(t0[:, 2 * BC:3 * BC],
                                                  xTw[0, :, 2 * BC:3 * BC])
                                nc.sync.dma_start(
                                    wts0[2][:], wf[0][0, 256:384, :])
                                nc.sync.dma_start(t0[:, 3 * BC:],
                                                  xTw[0, :, 3 * BC:])
                                nc.sync.dma_start(
                                    wts0[3][:], wf[0][0, 384:512, :])
                                for c in range(GW):
                                    nc.vector.tensor_tensor(
                                        xp[:, c * BC:(c + 1) * BC],
                                        t0[:, c * BC:(c + 1) * BC],
                                        brep_sb[0][:], ALU.mult)
                                    for j in range(n_ot):
                                        nc.tensor.matmul(
                                            ps[j][:],
                                            wts0[c][:, j * 128:(j + 1) * 128],
                                            xp[:, c * BC:(c + 1) * BC],
                                            start=(not with_bias and c == 0),
                                            stop=False)
                                load_brep_rest()
                                continue
                            if l == 0 and hT[g] is None:
                                load_x(g)
                            xp = xpool.tile([128, WIDE], MM_DT, tag="xp")
                            nc.vector.tensor_tensor(
                                xp[:].rearrange("p (c b) -> p c b", c=GW),
                                hT[g][:].rearrange("p (c b) -> p c b", c=GW),
                                brep_sb[e][:].unsqueeze(1).broadcast_to(
                                    (128, GW, BC)),
                                ALU.mult)
                            last_g = (e == E - 1 and g == NG - 1)
                            wts = []
                            for c in range(GW):
                                kt = g * GW + c
                                wt = wpool.tile([128, width], MM_DT, tag="wt")
                                row = e * I + kt * 128
                                nc.sync.dma_start(
                                    wt[:], wf[l][half_start // OH,
                                                 row:row + 128, :])
                                wts.append(wt)
                                opener = (not with_bias and e == 0 and g == 0
                                          and c == 0)
                                if not last_g:
                                    for j in range(n_ot):
                                        nc.tensor.matmul(
                                            ps[j][:], wt[:, j * 128:(j + 1) * 128],
                                            xp[:, c * BC:(c + 1) * BC],
                                            start=opener, stop=False)
                            if last_g:
                                # final group: j-outer so each bank's group
                                # stops early and its drain overlaps the rest
                                for j in range(n_ot):
                                    for c in range(GW):
                                        nc.tensor.matmul(
                                            ps[j][:], wts[c][:, j * 128:(j + 1) * 128],
                                            xp[:, c * BC:(c + 1) * BC],
                                            start=False, stop=(c == GW - 1))
                            # emit the previous half's drains only after this
                            # half's first waves, so the scheduler prioritizes
                            # restarting the PE pipeline; spread them over two
                            # waves so the DVE prescales stay interleaved
                            if g == 0 and e == 0 and pending_drain:
                                half = (len(pending_drain) + 1) // 2
                                for fn in pending_drain[:half]:
                                    fn()
                                pending_drain = pending_drain[half:]
                            elif g == 0 and e == 1 and pending_drain:
                                for fn in pending_drain:
                                    fn()
                                pending_drain = []
                    # defer drain emission (one closure per psum tile)
                    def make_drain(l, has_elu, half_start, j, ps_j, h_next):
                        def drain():
                            ot = (half_start + j * 128) // 128
                            if has_elu:
                                # elu(v) = relu(v) + exp(min(v,0)) - 1
                                m = tpool.tile([128, BC], dt.float32, tag="m")
                                nc.vector.tensor_scalar_min(m[:], ps_j[:], 0.0)
                                r = tpool.tile([128, BC], dt.float32, tag="r")
                                nc.scalar.activation(r[:], ps_j[:], ACTF.Relu)
                                x2 = tpool.tile([128, BC], dt.float32, tag="x2")
                                nc.scalar.activation(x2[:], m[:], ACTF.Exp)
                                dst = h_next[ot // GW][
                                    :, (ot % GW) * BC:(ot % GW + 1) * BC]
                                nc.vector.scalar_tensor_tensor(
                                    dst, x2[:], -1.0, r[:], ALU.add, ALU.add)
                            else:
                                y = ypool.tile([128, BC], dt.float32, tag="y")
                                nc.vector.tensor_copy(y[:], ps_j[:])
                                # one DMA per tile (a single transfer's
                                # packets already spread over all 16 DMA
                                # engines; splitting only adds ~600ns issue
                                # cost each); on the ACT queue so the store
                                # never queues behind weight DMAs
                                rows = slice(half_start + j * 128,
                                             half_start + (j + 1) * 128)
                                nc.scalar.dma_start(yT[rows, :], y[:])
                        return drain
                    for j in range(n_ot):
                        pending_drain.append(
                            make_drain(l, has_elu, half_start, j, ps[j], h_next))
                if has_elu:
                    hT = h_next
            for fn in pending_drain:
                fn()
    nc.compile()
    return nc


def _prep_inputs(weight_blend, x, W0, B0, W1, B1, W2, B2):
    Ws = [W0, W1, W2]
    Bs = [B0, B1, B2]
    shared = {}
    for l in range(3):
        I, O, _ = LAYERS[l]
        wfl = Ws[l].transpose(0, 2, 1).reshape(E * I, O)
        nh = max(O // OH, 1)
        shared[f"w{l}f"] = np.ascontiguousarray(
            np.stack([wfl[:, h * OH:h * OH + min(OH, O)] for h in range(nh)]),
            dtype=MM_NP)
        shared[f"b{l}f"] = np.ascontiguousarray(Bs[l][:, :, 0], dtype=MM_NP)
    in_maps = []
    for c in range(N_CORES):
        s = slice(c * BC, (c + 1) * BC)
        blT = np.ascontiguousarray(weight_blend[s].T, dtype=np.float32)
        m = dict(shared)
        # pack x.T into [NG, 128, GW*BC] wide tiles: block kt = g*GW + c
        xt = np.ascontiguousarray(x[s].T, dtype=np.float32)  # [1024, 512]
        m["xTw"] = np.ascontiguousarray(
            xt.reshape(-1, GW, 128, BC).transpose(0, 2, 1, 3).reshape(-1, 128, WIDE),
            dtype=MM_NP)
        m["blT"] = blT.astype(MM_NP)
        m["brep"] = np.ascontiguousarray(
            np.broadcast_to(blT[:, None, :], (E, 128, BC)), dtype=MM_NP)
        in_maps.append(m)
    return in_maps


def run(inputs, trace=False, tmpdir=None, trace_cores=None):
    """Run on hardware; returns (y, BassKernelResults)."""
    with_bias = any(
        np.any(np.asarray(inputs[k])) for k in ("B0", "B1", "B2"))
    key = ("nc", with_bias)
    if key not in _cache:
        _cache[key] = _build(with_bias)
    nc = _cache[key]
    in_maps = _prep_inputs(**inputs)
    kw = {}
    if tmpdir:
        kw["tmpdir"] = tmpdir
    if trace_cores:
        kw["trace_cores"] = trace_cores
    res = run_bass_kernel_spmd(
        nc, in_maps, core_ids=list(range(N_CORES)), trace=trace, **kw)
    y = np.concatenate([r["yT"].T for r in res.results], axis=0)
    return np.ascontiguousarray(y, dtype=np.float32), res


def kernel(**inputs):
    y, _ = run(inputs, trace=False)
    return y


# revision 13
# speedup vs baseline: 1.0002x; 1.0002x over previous
"""Blended-MoE 3-layer MLP (nn_Expert) on 8 Trainium2 NeuronCores.

Math: per layer, y[b,o] = act( sum_e blend[b,e] * (W[e] @ x[b] + B[e])[o] ).
Rewritten as a dense matmul with a per-expert prescale of the activations:
  y[o,b] = act( sum_e sum_k Wf[e*I+k, o] * (blend[e,b] * hT[k,b]) + bias )
where Wf[(e,i), o] = W[e,o,i] and everything is kept transposed on-chip
([feature, batch] layout) so each layer's output feeds the next directly.

Sharding: data-parallel over the batch — 4096 tokens -> 512 per core; the
per-expert weight stacks are replicated. Matmuls run in bf16 (issue interval
215.9ns vs 226.7ns for float32r — the fp32r path pays a fixed ~13ns/MM
instruction tax) with fp32 PSUM accumulation; rel err ~5e-3. PSUM accumulates
over experts and the contraction. Activations are stored 4 k-blocks wide
([128, 2048] bf16) so one DVE prescale op (with a free-dim-broadcast blend
operand, all-bf16 for the 2x 16-bit DVE rate) feeds 4 k-tiles of matmuls.
Weights are pre-split by o-half on the host so every tile DMA is one
contiguous 256KB block.

DMA issue is split across the two hardware-DGE queues: the Sync queue
carries only the weight stream (whose pool anti-dependency waits pace the
prefetch), while the Activation queue carries x/blend/bias loads and the
output stores, so none of those ever sit behind a stalled weight DMA
(head-of-line blocking on the in-order queue was worth ~20us at boot).
"""

import os

import numpy as np
import ml_dtypes

import concourse.bass as bass
import concourse.tile as tile
import concourse.mybir as mybir
from concourse import bacc
from concourse.bass_utils import run_bass_kernel_spmd
from contextlib import ExitStack

dt = mybir.dt
ALU = mybir.AluOpType
ACTF = mybir.ActivationFunctionType

N_CORES = 8
B_FULL = 4096
BC = B_FULL // N_CORES  # 512 tokens per core
E = 8
DIMS = [1024, 2048, 2048, 512]
LAYERS = [  # (I, O, has_elu)
    (1024, 2048, True),
    (2048, 2048, True),
    (2048, 512, False),
]
OH = 1024   # o-columns per half-pass (8 psum banks)
GW = 4      # k-blocks packed per wide activation tile
WIDE = GW * BC

USE_BF16 = os.environ.get("MOE_MM_DT", "bf16") == "bf16"
MM_DT = dt.bfloat16 if USE_BF16 else dt.float32r
MM_NP = ml_dtypes.bfloat16 if USE_BF16 else np.float32

_cache = {}


def _build(with_bias=True):
    nc = bacc.Bacc("TRN2", target_bir_lowering=False, debug=False,
                   num_devices=N_CORES)
    xTw = nc.declare_dram_parameter("xTw", [DIMS[0] // (128 * GW), 128, WIDE],
                                    MM_DT, isOutput=False)
    blT = nc.declare_dram_parameter("blT", [E, BC], MM_DT, isOutput=False)
    brep = nc.declare_dram_parameter("brep", [E, 128, BC], MM_DT, isOutput=False)
    # weights pre-split by o-half on the host so every [128, width] tile DMA
    # reads one fully contiguous block
    wf = [nc.declare_dram_parameter(f"w{l}f",
                                    [max(O // OH, 1), E * I, min(OH, O)],
                                    MM_DT, isOutput=False)
          for l, (I, O, _) in enumerate(LAYERS)]
    bf = [nc.declare_dram_parameter(f"b{l}f", [E, O], MM_DT, isOutput=False)
          for l, (I, O, _) in enumerate(LAYERS)]
    yT = nc.declare_dram_parameter("yT", [DIMS[3], BC], dt.float32, isOutput=True)

    tc = tile.TileContext(nc)
    with tc:
        with ExitStack() as ctx:
            const = ctx.enter_context(tc.tile_pool(name="const", bufs=1))
            act = ctx.enter_context(tc.tile_pool(name="act", bufs=1))
            xpool = ctx.enter_context(tc.tile_pool(name="xpool", bufs=4))
            wpool = ctx.enter_context(tc.tile_pool(name="wpool", bufs=24))
            tpool = ctx.enter_context(tc.tile_pool(name="tpool", bufs=2))
            ypool = ctx.enter_context(tc.tile_pool(name="ypool", bufs=4))
            pp = ctx.enter_context(tc.tile_pool(name="pp", bufs=8, space="PSUM"))

            if with_bias:
                blT_sb = const.tile([E, BC], MM_DT, tag="blT")
                nc.scalar.dma_start(blT_sb[:], blT[:])

            # warm the PE clock gate (HAM) with throwaway matmuls on a
            # memset constant (no DMA dependency) while the first input and
            # weight DMAs are in flight, so the real stream starts at
            # 2.4 GHz; the psum tile recycles into the first half's bank set
            if USE_BF16:
                wsrc = const.tile([E, BC], MM_DT, tag="wsrc")
                nc.vector.memset(wsrc[:], 1.0)
            else:
                wsrc_f = const.tile([E, BC], dt.float32, tag="wsrc_f")
                nc.vector.memset(wsrc_f[:], 1.0)
                wsrc = const.tile([E, BC], MM_DT, tag="wsrc")
                nc.vector.tensor_copy(wsrc[:], wsrc_f[:])
            warm = pp.tile([128, BC], dt.float32, tag="ps")
            for _ in range(12):
                nc.tensor.matmul(warm[:], wsrc[:, :128], wsrc[:],
                                 start=True, stop=True)

            # blend broadcast tiles: expert 0 first on the weight queue (it
            # gates the very first prescale), the rest trickled on the ACT
            # queue so they never queue behind weight DMAs
            brep_sb = [None] * E
            brep_sb[0] = const.tile([128, BC], MM_DT, name="brep0", tag="brep0")
            nc.sync.dma_start(brep_sb[0][:], brep[0, :, :])

            def load_brep_rest():
                for e in range(1, E):
                    t = const.tile([128, BC], MM_DT, name=f"brep{e}",
                                   tag=f"brep{e}")
                    nc.scalar.dma_start(t[:], brep[e, :, :])
                    brep_sb[e] = t

            bf_sb = [None, None, None]

            # layer 0 input, packed 4 k-blocks wide; tile g loaded lazily at
            # first use so startup only waits for tile 0 (which is itself
            # split into chunks so the first prescale can begin early)
            hT = [None] * (DIMS[0] // (128 * GW))

            def load_x(g):
                t = act.tile([128, WIDE], MM_DT, name=f"h0_{g}", tag=f"h0_{g}")
                # 2 chunks spread across DMA engines for latency
                nc.scalar.dma_start(t[:, :WIDE // 2], xTw[g, :, :WIDE // 2])
                nc.scalar.dma_start(t[:, WIDE // 2:], xTw[g, :, WIDE // 2:])
                hT[g] = t

            pending_drain = []  # deferred drain emission from the previous half

            for l, (I, O, has_elu) in enumerate(LAYERS):
                NG = I // (128 * GW)  # wide groups per layer input
                if with_bias:
                    t = const.tile([E, O], MM_DT, tag=f"bf{l}")
                    nc.scalar.dma_start(t[:], bf[l][:])
                    bf_sb[l] = t
                h_next = []
                if has_elu:
                    for g in range(O * BC // (128 * WIDE)):
                        h_next.append(act.tile([128, WIDE], MM_DT,
                                               name=f"h{l + 1}_{g}",
                                               tag=f"h{l + 1}_{g}"))
                for half_start in range(0, O, OH):
                    width = min(OH, O - half_start)
                    n_ot = width // 128
                    # open accumulation groups with the bias matmul (K=8);
                    # without bias the first weight matmul opens the group
                    ps = []
                    for j in range(n_ot):
                        p = pp.tile([128, BC], dt.float32, tag="ps")
                        if with_bias:
                            nc.tensor.matmul(
                                p[:],
                                bf_sb[l][:, half_start + j * 128:
                                         half_start + (j + 1) * 128],
                                blT_sb[:],
                                start=True, stop=False)
                        ps.append(p)
                    # stream wide k-groups x experts: g-outer so the second
                    # x group / later brep tiles aren't needed until hundreds
                    # of matmuls in (the e-outer order starved the boot)
                    for g in range(NG):
                        for e in range(E):
                            if l == 0 and half_start == 0 and e == 0 and g == 0:
                                # boot: cold DMA completion latency is ~3.5us,
                                # so issue every boot-critical transfer up
                                # front, ordered by when the pipeline needs
                                # it (x_c0, first weights, then alternating),
                                # and bridge the wait with the warmup matmuls
                                t0 = act.tile([128, WIDE], MM_DT,
                                              name="h0_0", tag="h0_0")
                                hT[0] = t0
                                xp = xpool.tile([128, WIDE], MM_DT, tag="xp")
                                wts0 = [wpool.tile([128, width], MM_DT,
                                                   name=f"wtb{c}", tag="wt")
                                        for c in range(GW)]
                                nc.sync.dma_start(t0[:, :BC], xTw[0, :, :BC])
                                hw = width // 2
                                nc.sync.dma_start(
                                    wts0[0][:, :hw], wf[0][0, :128, :hw])
                                nc.sync.dma_start(
                                    wts0[0][:, hw:], wf[0][0, :128, hw:])
                                nc.sync.dma_start(t0[:, BC:2 * BC],
                                                  xTw[0, :, BC:2 * BC])
                                nc.sync.dma_start(
                                    wts0[1][:], wf[0][0, 128:256, :])
                                nc.sync.dma_start(t0[:, 2 * BC:3 * BC],
                                                  xTw[0, :, 2 * BC:3 * BC])
                                nc.sync.dma_start(
                                    wts0[2][:], wf[0][0, 256:384, :])
                                nc.sync.dma_start(t0[:, 3 * BC:],
                                                  xTw[0, :, 3 * BC:])
                                nc.sync.dma_start(
                                    wts0[3][:], wf[0][0, 384:512, :])
                                for c in range(GW):
                                    nc.vector.tensor_tensor(
                                        xp[:, c * BC:(c + 1) * BC],
                                        t0[:, c * BC:(c + 1) * BC],
                                        brep_sb[0][:], ALU.mult)
                                    for j in range(n_ot):
                                        nc.tensor.matmul(
                                            ps[j][:],
                                            wts0[c][:, j * 128:(j + 1) * 128],
                                            xp[:, c * BC:(c + 1) * BC],
                                            start=(not with_bias and c == 0),
                                            stop=False)
                                load_brep_rest()
                                continue
                            if l == 0 and hT[g] is None:
                                load_x(g)
                            xp = xpool.tile([128, WIDE], MM_DT, tag="xp")
                            nc.vector.tensor_tensor(
                                xp[:].rearrange("p (c b) -> p c b", c=GW),
                                hT[g][:].rearrange("p (c b) -> p c b", c=GW),
                                brep_sb[e][:].unsqueeze(1).broadcast_to(
                                    (128, GW, BC)),
                                ALU.mult)
                            last_g = (e == E - 1 and g == NG - 1)
                            wts = []
                            for c in range(GW):
                                kt = g * GW + c
                                wt = wpool.tile([128, width], MM_DT, tag="wt")
                                row = e * I + kt * 128
                                nc.sync.dma_start(
                                    wt[:], wf[l][half_start // OH,
                                                 row:row + 128, :])
                                wts.append(wt)
                                opener = (not with_bias and e == 0 and g == 0
                                          and c == 0)
                                if not last_g:
                                    for j in range(n_ot):
                                        nc.tensor.matmul(
                                            ps[j][:], wt[:, j * 128:(j + 1) * 128],
                                            xp[:, c * BC:(c + 1) * BC],
                                            start=opener, stop=False)
                            if last_g:
                                # final group: j-outer so each bank's group
                                # stops early and its drain overlaps the rest
                                for j in range(n_ot):
                                    for c in range(GW):
                                        nc.tensor.matmul(
                                            ps[j][:], wts[c][:, j * 128:(j + 1) * 128],
                                            xp[:, c * BC:(c + 1) * BC],
                                            start=False, stop=(c == GW - 1))
                            # emit the previous half's drains only after this
                            # half's first waves, so the scheduler prioritizes
                            # restarting the PE pipeline; spread them over two
                            # waves so the DVE prescales stay interleaved
                            if g == 0 and e == 0 and pending_drain:
                                half = (len(pending_drain) + 1) // 2
                                for fn in pending_drain[:half]:
                                    fn()
                                pending_drain = pending_drain[half:]
                            elif g == 0 and e == 1 and pending_drain:
                                for fn in pending_drain:
                                    fn()
                                pending_drain = []
                    # defer drain emission (one closure per psum tile)
                    def make_drain(l, has_elu, half_start, j, ps_j, h_next):
                        def drain():
                            ot = (half_start + j * 128) // 128
                            if has_elu:
                                # elu(v) = relu(v) + exp(min(v,0)) - 1
                                m = tpool.tile([128, BC], dt.float32, tag="m")
                                nc.vector.tensor_scalar_min(m[:], ps_j[:], 0.0)
                                r = tpool.tile([128, BC], dt.float32, tag="r")
                                nc.scalar.activation(r[:], ps_j[:], ACTF.Relu)
                                x2 = tpool.tile([128, BC], dt.float32, tag="x2")
                                nc.scalar.activation(x2[:], m[:], ACTF.Exp)
                                dst = h_next[ot // GW][
                                    :, (ot % GW) * BC:(ot % GW + 1) * BC]
                                nc.vector.scalar_tensor_tensor(
                                    dst, x2[:], -1.0, r[:], ALU.add, ALU.add)
                            else:
                                y = ypool.tile([128, BC], dt.float32, tag="y")
                                nc.vector.tensor_copy(y[:], ps_j[:])
                                # one DMA per tile (a single transfer's
                                # packets already spread over all 16 DMA
                                # engines; splitting only adds ~600ns issue
                                # cost each); on the ACT queue so the store
                                # never queues behind weight DMAs
                                rows = slice(half_start + j * 128,
                                             half_start + (j + 1) * 128)
                                nc.scalar.dma_start(yT[rows, :], y[:])
                        return drain
                    for j in range(n_ot):
                        pending_drain.append(
                            make_drain(l, has_elu, half_start, j, ps[j], h_next))
                if has_elu:
                    hT = h_next
            for fn in pending_drain:
                fn()
    nc.compile()
    return nc


def _prep_inputs(weight_blend, x, W0, B0, W1, B1, W2, B2):
    Ws = [W0, W1, W2]
    Bs = [B0, B1, B2]
    shared = {}
    for l in range(3):
        I, O, _ = LAYERS[l]
        wfl = Ws[l].transpose(0, 2, 1).reshape(E * I, O)
        nh = max(O // OH, 1)
        shared[f"w{l}f"] = np.ascontiguousarray(
            np.stack([wfl[:, h * OH:h * OH + min(OH, O)] for h in range(nh)]),
            dtype=MM_NP)
        shared[f"b{l}f"] = np.ascontiguousarray(Bs[l][:, :, 0], dtype=MM_NP)
    in_maps = []
    for c in range(N_CORES):
        s = slice(c * BC, (c + 1) * BC)
        blT = np.ascontiguousarray(weight_blend[s].T, dtype=np.float32)
        m = dict(shared)
        # pack x.T into [NG, 128, GW*BC] wide tiles: block kt = g*GW + c
        xt = np.ascontiguousarray(x[s].T, dtype=np.float32)  # [1024, 512]
        m["xTw"] = np.ascontiguousarray(
            xt.reshape(-1, GW, 128, BC).transpose(0, 2, 1, 3).reshape(-1, 128, WIDE),
            dtype=MM_NP)
        m["blT"] = blT.astype(MM_NP)
        m["brep"] = np.ascontiguousarray(
            np.broadcast_to(blT[:, None, :], (E, 128, BC)), dtype=MM_NP)
        in_maps.append(m)
    return in_maps


def run(inputs, trace=False, tmpdir=None, trace_cores=None):
    """Run on hardware; returns (y, BassKernelResults)."""
    with_bias = any(
        np.any(np.asarray(inputs[k])) for k in ("B0", "B1", "B2"))
    key = ("nc", with_bias)
    if key not in _cache:
        _cache[key] = _build(with_bias)
    nc = _cache[key]
    in_maps = _prep_inputs(**inputs)
    kw = {}
    if tmpdir:
        kw["tmpdir"] = tmpdir
    if trace_cores:
        kw["trace_cores"] = trace_cores
    res = run_bass_kernel_spmd(
        nc, in_maps, core_ids=list(range(N_CORES)), trace=trace, **kw)
    y = np.concatenate([r["yT"].T for r in res.results], axis=0)
    return np.ascontiguousarray(y, dtype=np.float32), res


def kernel(**inputs):
    y, _ = run(inputs, trace=False)
    return y


# revision 14
# speedup vs baseline: 1.0006x; 1.0005x over previous
"""Blended-MoE 3-layer MLP (nn_Expert) on 8 Trainium2 NeuronCores.

Math: per layer, y[b,o] = act( sum_e blend[b,e] * (W[e] @ x[b] + B[e])[o] ).
Rewritten as a dense matmul with a per-expert prescale of the activations:
  y[o,b] = act( sum_e sum_k Wf[e*I+k, o] * (blend[e,b] * hT[k,b]) + bias )
where Wf[(e,i), o] = W[e,o,i] and everything is kept transposed on-chip
([feature, batch] layout) so each layer's output feeds the next directly.

Sharding: data-parallel over the batch — 4096 tokens -> 512 per core; the
per-expert weight stacks are replicated. Matmuls run in bf16 (issue interval
215.9ns vs 226.7ns for float32r — the fp32r path pays a fixed ~13ns/MM
instruction tax) with fp32 PSUM accumulation; rel err ~5e-3. PSUM accumulates
over experts and the contraction. Activations are stored 4 k-blocks wide
([128, 2048] bf16) so one DVE prescale op (with a free-dim-broadcast blend
operand, all-bf16 for the 2x 16-bit DVE rate) feeds 4 k-tiles of matmuls.
Weights are pre-split by o-half on the host so every tile DMA is one
contiguous 256KB block.

DMA issue is split across the two hardware-DGE queues: the Sync queue
carries only the weight stream (whose pool anti-dependency waits pace the
prefetch), while the Activation queue carries x/blend/bias loads and the
output stores, so none of those ever sit behind a stalled weight DMA
(head-of-line blocking on the in-order queue was worth ~20us at boot).
"""

import os

import numpy as np
import ml_dtypes

import concourse.bass as bass
import concourse.tile as tile
import concourse.mybir as mybir
from concourse import bacc
from concourse.bass_utils import run_bass_kernel_spmd
from contextlib import ExitStack

dt = mybir.dt
ALU = mybir.AluOpType
ACTF = mybir.ActivationFunctionType

N_CORES = 8
B_FULL = 4096
BC = B_FULL // N_CORES  # 512 tokens per core
E = 8
DIMS = [1024, 2048, 2048, 512]
LAYERS = [  # (I, O, has_elu)
    (1024, 2048, True),
    (2048, 2048, True),
    (2048, 512, False),
]
OH = 1024   # o-columns per half-pass (8 psum banks)
GW = 4      # k-blocks packed per wide activation tile
WIDE = GW * BC

USE_BF16 = os.environ.get("MOE_MM_DT", "bf16") == "bf16"
MM_DT = dt.bfloat16 if USE_BF16 else dt.float32r
MM_NP = ml_dtypes.bfloat16 if USE_BF16 else np.float32

_cache = {}


def _build(with_bias=True):
    nc = bacc.Bacc("TRN2", target_bir_lowering=False, debug=False,
                   num_devices=N_CORES)
    xTw = nc.declare_dram_parameter("xTw", [DIMS[0] // (128 * GW), 128, WIDE],
                                    MM_DT, isOutput=False)
    blT = nc.declare_dram_parameter("blT", [E, BC], MM_DT, isOutput=False)
    brep = nc.declare_dram_parameter("brep", [E, 128, BC], MM_DT, isOutput=False)
    # weights pre-split by o-half on the host so every [128, width] tile DMA
    # reads one fully contiguous block
    wf = [nc.declare_dram_parameter(f"w{l}f",
                                    [max(O // OH, 1), E * I, min(OH, O)],
                                    MM_DT, isOutput=False)
          for l, (I, O, _) in enumerate(LAYERS)]
    bf = [nc.declare_dram_parameter(f"b{l}f", [E, O], MM_DT, isOutput=False)
          for l, (I, O, _) in enumerate(LAYERS)]
    yT = nc.declare_dram_parameter("yT", [DIMS[3], BC], dt.float32, isOutput=True)

    tc = tile.TileContext(nc)
    with tc:
        with ExitStack() as ctx:
            const = ctx.enter_context(tc.tile_pool(name="const", bufs=1))
            act = ctx.enter_context(tc.tile_pool(name="act", bufs=1))
            xpool = ctx.enter_context(tc.tile_pool(name="xpool", bufs=4))
            wpool = ctx.enter_context(tc.tile_pool(name="wpool", bufs=24))
            tpool = ctx.enter_context(tc.tile_pool(name="tpool", bufs=2))
            ypool = ctx.enter_context(tc.tile_pool(name="ypool", bufs=4))
            pp = ctx.enter_context(tc.tile_pool(name="pp", bufs=8, space="PSUM"))

            if with_bias:
                blT_sb = const.tile([E, BC], MM_DT, tag="blT")
                nc.scalar.dma_start(blT_sb[:], blT[:])

            # warm the PE clock gate (HAM) with throwaway matmuls on a
            # memset constant (no DMA dependency) while the first input and
            # weight DMAs are in flight, so the real stream starts at
            # 2.4 GHz; the psum tile recycles into the first half's bank set
            # warmups must be FULL-K (128 partitions): the HAM activity
            # monitor tracks PE-cell busyness, and a K=8 matmul lights only
            # 6% of the array — it never flips the clock gate to 2.4 GHz
            if USE_BF16:
                wsrc = const.tile([128, BC], MM_DT, tag="wsrc")
                nc.vector.memset(wsrc[:], 1.0)
            else:
                wsrc_f = const.tile([128, BC], dt.float32, tag="wsrc_f")
                nc.vector.memset(wsrc_f[:], 1.0)
                wsrc = const.tile([128, BC], MM_DT, tag="wsrc")
                nc.vector.tensor_copy(wsrc[:], wsrc_f[:])
            warm = pp.tile([128, BC], dt.float32, tag="ps")
            for _ in range(12):
                nc.tensor.matmul(warm[:], wsrc[:, :128], wsrc[:],
                                 start=True, stop=True)

            # blend broadcast tiles: expert 0 first on the weight queue (it
            # gates the very first prescale), the rest trickled on the ACT
            # queue so they never queue behind weight DMAs
            brep_sb = [None] * E
            brep_sb[0] = const.tile([128, BC], MM_DT, name="brep0", tag="brep0")
            nc.sync.dma_start(brep_sb[0][:], brep[0, :, :])

            def load_brep_rest():
                for e in range(1, E):
                    t = const.tile([128, BC], MM_DT, name=f"brep{e}",
                                   tag=f"brep{e}")
                    nc.scalar.dma_start(t[:], brep[e, :, :])
                    brep_sb[e] = t

            bf_sb = [None, None, None]

            # layer 0 input, packed 4 k-blocks wide; tile g loaded lazily at
            # first use so startup only waits for tile 0 (which is itself
            # split into chunks so the first prescale can begin early)
            hT = [None] * (DIMS[0] // (128 * GW))

            def load_x(g):
                t = act.tile([128, WIDE], MM_DT, name=f"h0_{g}", tag=f"h0_{g}")
                # 2 chunks spread across DMA engines for latency
                nc.scalar.dma_start(t[:, :WIDE // 2], xTw[g, :, :WIDE // 2])
                nc.scalar.dma_start(t[:, WIDE // 2:], xTw[g, :, WIDE // 2:])
                hT[g] = t

            pending_drain = []  # deferred drain emission from the previous half

            for l, (I, O, has_elu) in enumerate(LAYERS):
                NG = I // (128 * GW)  # wide groups per layer input
                if with_bias:
                    t = const.tile([E, O], MM_DT, tag=f"bf{l}")
                    nc.scalar.dma_start(t[:], bf[l][:])
                    bf_sb[l] = t
                h_next = []
                if has_elu:
                    for g in range(O * BC // (128 * WIDE)):
                        h_next.append(act.tile([128, WIDE], MM_DT,
                                               name=f"h{l + 1}_{g}",
                                               tag=f"h{l + 1}_{g}"))
                for half_start in range(0, O, OH):
                    width = min(OH, O - half_start)
                    n_ot = width // 128
                    # open accumulation groups with the bias matmul (K=8);
                    # without bias the first weight matmul opens the group
                    ps = []
                    for j in range(n_ot):
                        p = pp.tile([128, BC], dt.float32, tag="ps")
                        if with_bias:
                            nc.tensor.matmul(
                                p[:],
                                bf_sb[l][:, half_start + j * 128:
                                         half_start + (j + 1) * 128],
                                blT_sb[:],
                                start=True, stop=False)
                        ps.append(p)
                    # stream wide k-groups x experts: g-outer so the second
                    # x group / later brep tiles aren't needed until hundreds
                    # of matmuls in (the e-outer order starved the boot)
                    for g in range(NG):
                        for e in range(E):
                            if l == 0 and half_start == 0 and e == 0 and g == 0:
                                # boot: cold DMA completion latency is ~3.5us,
                                # so issue every boot-critical transfer up
                                # front, ordered by when the pipeline needs
                                # it (x_c0, first weights, then alternating),
                                # and bridge the wait with the warmup matmuls
                                t0 = act.tile([128, WIDE], MM_DT,
                                              name="h0_0", tag="h0_0")
                                hT[0] = t0
                                xp = xpool.tile([128, WIDE], MM_DT, tag="xp")
                                wts0 = [wpool.tile([128, width], MM_DT,
                                                   name=f"wtb{c}", tag="wt")
                                        for c in range(GW)]
                                nc.sync.dma_start(t0[:, :BC], xTw[0, :, :BC])
                                hw = width // 2
                                nc.sync.dma_start(
                                    wts0[0][:, :hw], wf[0][0, :128, :hw])
                                nc.sync.dma_start(
                                    wts0[0][:, hw:], wf[0][0, :128, hw:])
                                nc.sync.dma_start(t0[:, BC:2 * BC],
                                                  xTw[0, :, BC:2 * BC])
                                nc.sync.dma_start(
                                    wts0[1][:], wf[0][0, 128:256, :])
                                nc.sync.dma_start(t0[:, 2 * BC:3 * BC],
                                                  xTw[0, :, 2 * BC:3 * BC])
                                nc.sync.dma_start(
                                    wts0[2][:], wf[0][0, 256:384, :])
                                nc.sync.dma_start(t0[:, 3 * BC:],
                                                  xTw[0, :, 3 * BC:])
                                nc.sync.dma_start(
                                    wts0[3][:], wf[0][0, 384:512, :])
                                for c in range(GW):
                                    nc.vector.tensor_tensor(
                                        xp[:, c * BC:(c + 1) * BC],
                                        t0[:, c * BC:(c + 1) * BC],
                                        brep_sb[0][:], ALU.mult)
                                    for j in range(n_ot):
                                        nc.tensor.matmul(
                                            ps[j][:],
                                            wts0[c][:, j * 128:(j + 1) * 128],
                                            xp[:, c * BC:(c + 1) * BC],
                                            start=(not with_bias and c == 0),
                                            stop=False)
                                load_brep_rest()
                                continue
                            if l == 0 and hT[g] is None:
                                load_x(g)
                            xp = xpool.tile([128, WIDE], MM_DT, tag="xp")
                            nc.vector.tensor_tensor(
                                xp[:].rearrange("p (c b) -> p c b", c=GW),
                                hT[g][:].rearrange("p (c b) -> p c b", c=GW),
                                brep_sb[e][:].unsqueeze(1).broadcast_to(
                                    (128, GW, BC)),
                                ALU.mult)
                            last_g = (e == E - 1 and g == NG - 1)
                            wts = []
                            for c in range(GW):
                                kt = g * GW + c
                                wt = wpool.tile([128, width], MM_DT, tag="wt")
                                row = e * I + kt * 128
                                nc.sync.dma_start(
                                    wt[:], wf[l][half_start // OH,
                                                 row:row + 128, :])
                                wts.append(wt)
                                opener = (not with_bias and e == 0 and g == 0
                                          and c == 0)
                                if not last_g:
                                    for j in range(n_ot):
                                        nc.tensor.matmul(
                                            ps[j][:], wt[:, j * 128:(j + 1) * 128],
                                            xp[:, c * BC:(c + 1) * BC],
                                            start=opener, stop=False)
                            if last_g:
                                # final group: j-outer so each bank's group
                                # stops early and its drain overlaps the rest
                                for j in range(n_ot):
                                    for c in range(GW):
                                        nc.tensor.matmul(
                                            ps[j][:], wts[c][:, j * 128:(j + 1) * 128],
                                            xp[:, c * BC:(c + 1) * BC],
                                            start=False, stop=(c == GW - 1))
                            # emit the previous half's drains only after this
                            # half's first waves, so the scheduler prioritizes
                            # restarting the PE pipeline; spread them over two
                            # waves so the DVE prescales stay interleaved
                            if g == 0 and e == 0 and pending_drain:
                                half = (len(pending_drain) + 1) // 2
                                for fn in pending_drain[:half]:
                                    fn()
                                pending_drain = pending_drain[half:]
                            elif g == 0 and e == 1 and pending_drain:
                                for fn in pending_drain:
                                    fn()
                                pending_drain = []
                    # defer drain emission (one closure per psum tile)
                    def make_drain(l, has_elu, half_start, j, ps_j, h_next):
                        def drain():
                            ot = (half_start + j * 128) // 128
                            if has_elu:
                                # elu(v) = relu(v) + exp(min(v,0)) - 1
                                m = tpool.tile([128, BC], dt.float32, tag="m")
                                nc.vector.tensor_scalar_min(m[:], ps_j[:], 0.0)
                                r = tpool.tile([128, BC], dt.float32, tag="r")
                                nc.scalar.activation(r[:], ps_j[:], ACTF.Relu)
                                x2 = tpool.tile([128, BC], dt.float32, tag="x2")
                                nc.scalar.activation(x2[:], m[:], ACTF.Exp)
                                dst = h_next[ot // GW][
                                    :, (ot % GW) * BC:(ot % GW + 1) * BC]
                                nc.vector.scalar_tensor_tensor(
                                    dst, x2[:], -1.0, r[:], ALU.add, ALU.add)
                            else:
                                y = ypool.tile([128, BC], dt.float32, tag="y")
                                nc.vector.tensor_copy(y[:], ps_j[:])
                                # one DMA per tile (a single transfer's
                                # packets already spread over all 16 DMA
                                # engines; splitting only adds ~600ns issue
                                # cost each); on the ACT queue so the store
                                # never queues behind weight DMAs
                                rows = slice(half_start + j * 128,
                                             half_start + (j + 1) * 128)
                                nc.scalar.dma_start(yT[rows, :], y[:])
                        return drain
                    for j in range(n_ot):
                        pending_drain.append(
                            make_drain(l, has_elu, half_start, j, ps[j], h_next))
                if has_elu:
                    hT = h_next
            for fn in pending_drain:
                fn()
    nc.compile()
    return nc


def _prep_inputs(weight_blend, x, W0, B0, W1, B1, W2, B2):
    Ws = [W0, W1, W2]
    Bs = [B0, B1, B2]
    shared = {}
    for l in range(3):
        I, O, _ = LAYERS[l]
        wfl = Ws[l].transpose(0, 2, 1).reshape(E * I, O)
        nh = max(O // OH, 1)
        shared[f"w{l}f"] = np.ascontiguousarray(
            np.stack([wfl[:, h * OH:h * OH + min(OH, O)] for h in range(nh)]),
            dtype=MM_NP)
        shared[f"b{l}f"] = np.ascontiguousarray(Bs[l][:, :, 0], dtype=MM_NP)
    in_maps = []
    for c in range(N_CORES):
        s = slice(c * BC, (c + 1) * BC)
        blT = np.ascontiguousarray(weight_blend[s].T, dtype=np.float32)
        m = dict(shared)
        # pack x.T into [NG, 128, GW*BC] wide tiles: block kt = g*GW + c
        xt = np.ascontiguousarray(x[s].T, dtype=np.float32)  # [1024, 512]
        m["xTw"] = np.ascontiguousarray(
            xt.reshape(-1, GW, 128, BC).transpose(0, 2, 1, 3).reshape(-1, 128, WIDE),
            dtype=MM_NP)
        m["blT"] = blT.astype(MM_NP)
        m["brep"] = np.ascontiguousarray(
            np.broadcast_to(blT[:, None, :], (E, 128, BC)), dtype=MM_NP)
        in_maps.append(m)
    return in_maps


def run(inputs, trace=False, tmpdir=None, trace_cores=None):
    """Run on hardware; returns (y, BassKernelResults)."""
    with_bias = any(
        np.any(np.asarray(inputs[k])) for k in ("B0", "B1", "B2"))
    key = ("nc", with_bias)
    if key not in _cache:
        _cache[key] = _build(with_bias)
    nc = _cache[key]
    in_maps = _prep_inputs(**inputs)
    kw = {}
    if tmpdir:
        kw["tmpdir"] = tmpdir
    if trace_cores:
        kw["trace_cores"] = trace_cores
    res = run_bass_kernel_spmd(
        nc, in_maps, core_ids=list(range(N_CORES)), trace=trace, **kw)
    y = np.concatenate([r["yT"].T for r in res.results], axis=0)
    return np.ascontiguousarray(y, dtype=np.float32), res


def kernel(**inputs):
    y, _ = run(inputs, trace=False)
    return y
